# revision 1
# baseline (speedup 1.0000x reference)
"""MetapathAggrLayer Trainium2 kernel — v2 (custom DVE ops).

Per node n: e_m = leakyrelu(x[m,n,:].a), w = softmax(e), out = sum_m w_m x[m,n,:].
Data-parallel over N across 8 NeuronCores; nodes-on-partitions layout.

v2: scores via a fused multiply+prefix-scan custom DVE op (segment sums
recovered as prefix differences at chunk boundaries), weighted sum via a
dual-MAC custom op (x0*w0 + x1*w1 per instruction), pair-combine adds on
GpSimd to offload the Vector engine.
"""

import sys

sys.path.insert(0, "/opt/trn_rl_repo")

import numpy as np

import concourse.bacc as bacc
import concourse.mybir as mybir
from concourse import bass_utils, dve_ops
from concourse.dve_spec import Spec, Src0, Src1, C0, C1, scan, maxx, AluOp, lower, _has_src1
from concourse.dve_uop import DveOpSpec
from concourse.tile import TileContext

ALPHA = 0.2
NMETA = 4
F = 64
N_FULL = 1_000_000
N_CORES = 8
T = 16                     # chunks (nodes per partition) per macro-tile
NODES_PER_MACRO = 128 * T  # 2048
MACROS_PER_CORE = 62
NC_NODES = MACROS_PER_CORE * NODES_PER_MACRO  # 126_976
N_PAD = N_CORES * NC_NODES                    # 1_015_808

MAC_ADD_ENGINE = "gpsimd"  # "gpsimd" | "vector"

_CACHE = {}


def _register_op(name, spec, subdim=False):
    if name in dve_ops._SUB_OPCODE_FOR_NAME:
        return next(o for o in dve_ops.OPS if o.name == name)
    row = dve_ops._CUSTOM_DVE_ROW_BASE + len(dve_ops.OPS)
    assert row < 0x20
    shas = {}
    for ver in ("v3", "v4"):
        s = DveOpSpec(name=name, opcode=row, uops=lower(spec, ver=ver),
                      rd1_en=_has_src1(spec))
        shas[ver] = s.sha(ver)
    op = dve_ops.DveOp(name, spec, subdim, shas)
    dve_ops.OPS.append(op)
    dve_ops.CUSTOM_DVE_SPECS[name] = spec
    dve_ops._SUB_OPCODE_FOR_NAME[name] = row
    return op


def _get_ops():
    scan_mul = _register_op(
        "MPA_SCAN_MUL",
        Spec(
            body=scan(AluOp.ADD, Src0 * Src1),
            reference=lambda in0, in1, s0, s1: np.cumsum(
                (in0.astype(np.float32) * in1.astype(np.float32)), axis=-1
            ),
        ),
    )
    ext_lrelu = _register_op(
        "MPA_EXT_LRELU",
        Spec(
            body=(lambda d: maxx(d, d * C0))(Src0 - Src1),
            reference=lambda in0, in1, s0, s1: np.maximum(in0 - in1, (in0 - in1) * s0),
        ),
    )
    dual_mac = _register_op(
        "MPA_DUAL_MAC",
        Spec(
            body=Src0 * C0 + Src1 * C1,
            reference=lambda in0, in1, s0, s1: in0 * s0 + in1 * s1,
        ),
    )
    return scan_mul, dual_mac, ext_lrelu


def _build_kernel():
    scan_mul, dual_mac, ext_lrelu = _get_ops()

    nc = bacc.Bacc("TRN2", target_bir_lowering=False, debug=False)
    dt = mybir.dt.float32

    x_in = nc.dram_tensor("input", (NMETA, NC_NODES, F), dt, kind="ExternalInput").ap()
    a_rep_in = nc.dram_tensor("a_rep", (128, T * F), dt, kind="ExternalInput").ap()
    out = nc.dram_tensor("out", (NC_NODES, F), dt, kind="ExternalOutput").ap()

    mult = mybir.AluOpType.mult
    add = mybir.AluOpType.add
    subtract = mybir.AluOpType.subtract
    op_max = mybir.AluOpType.max

    with TileContext(nc) as tc:
        with tc.tile_pool(name="const", bufs=1) as cpool, \
             tc.tile_pool(name="sbuf", bufs=3) as pool, \
             tc.tile_pool(name="scratch", bufs=2) as scpool, \
             tc.tile_pool(name="small", bufs=4) as spool:
            a_rep = cpool.tile([128, T * F], dt)
            nc.sync.dma_start(out=a_rep[:, :], in_=a_rep_in)

            for i in range(MACROS_PER_CORE):
                lo = i * NODES_PER_MACRO
                hi = lo + NODES_PER_MACRO

                xt = []
                for m in range(NMETA):
                    src = x_in[m, lo:hi, :].rearrange("(p t) f -> p (t f)", p=128)
                    xm = pool.tile([128, T * F], dt, tag=f"x{m}")
                    nc.sync.dma_start(out=xm[:, :], in_=src)
                    xt.append(xm)

                # ---- scores: prefix scan of x*a, segment sums by differencing
                e = spool.tile([128, NMETA * T], dt, tag="e")
                for m in range(NMETA):
                    pm = scpool.tile([128, T * F + 1], dt, tag=f"P{m}")
                    nc.gpsimd.memset(pm[:, 0:1], 0.0)
                    nc.vector._custom_dve(
                        scan_mul, out=pm[:, 1:T * F + 1],
                        in0=xt[m][:, :], in1=a_rep[:, :],
                    )
                    p_hi = pm[:, 1:T * F + 1].rearrange(
                        "p (t f) -> p t f", f=F)[:, :, F - 1:F]
                    p_lo = pm[:, 0:T * F].rearrange(
                        "p (t f) -> p t f", f=F)[:, :, 0:1]
                    nc.vector.tensor_tensor(
                        out=e[:, m * T:(m + 1) * T], in0=p_hi, in1=p_lo, op=subtract
                    )

                # ---- leakyrelu on DVE, exp on ScalarE
                u = spool.tile([128, NMETA * T], dt, tag="u")
                et = spool.tile([128, NMETA * T], dt, tag="et")
                nc.vector.tensor_scalar_mul(et[:, :], e[:, :], ALPHA)
                nc.vector.tensor_tensor(out=et[:, :], in0=e[:, :], in1=et[:, :], op=op_max)
                nc.scalar.activation(u[:, :], et[:, :], mybir.ActivationFunctionType.Exp)

                # ---- s = sum_m u_m ; r = 1/s ; w_m = u_m * r
                s01 = spool.tile([128, T], dt, tag="s01")
                s23 = spool.tile([128, T], dt, tag="s23")
                s = spool.tile([128, T], dt, tag="s")
                nc.vector.tensor_tensor(out=s01[:, :], in0=u[:, 0:T], in1=u[:, T:2 * T], op=add)
                nc.vector.tensor_tensor(out=s23[:, :], in0=u[:, 2 * T:3 * T], in1=u[:, 3 * T:4 * T], op=add)
                nc.vector.tensor_tensor(out=s[:, :], in0=s01[:, :], in1=s23[:, :], op=add)
                r = spool.tile([128, T], dt, tag="r")
                nc.vector.reciprocal(r[:, :], s[:, :])
                w = spool.tile([128, NMETA * T], dt, tag="w")
                r_bc = r[:, :].rearrange("p (o t) -> p o t", o=1).broadcast_to(
                    [128, NMETA, T])
                u_3d = u[:, :].rearrange("p (m t) -> p m t", m=NMETA)
                w_3d = w[:, :].rearrange("p (m t) -> p m t", m=NMETA)
                nc.vector.tensor_tensor(out=w_3d, in0=u_3d, in1=r_bc, op=mult)

                # ---- weighted sum: pair (0,1) dual-MAC on DVE; metapaths 2,3
                # scaled on ScalarE (activation Copy, per-partition scale);
                # combined with two full-width GpSimd adds.
                acc = scpool.tile([128, T * F], dt, tag="acc")
                acc1 = scpool.tile([128, T * F], dt, tag="acc1")
                t01 = scpool.tile([128, T * F], dt, tag="t01")
                t2 = scpool.tile([128, T * F], dt, tag="t2")
                t3 = scpool.tile([128, T * F], dt, tag="t3")
                for t in range(T):
                    fs = t * F
                    nc.vector._custom_dve(
                        dual_mac, out=t01[:, fs:fs + F],
                        in0=xt[0][:, fs:fs + F], in1=xt[1][:, fs:fs + F],
                        s0=w[:, t:t + 1], s1=w[:, T + t:T + t + 1],
                    )
                    nc.scalar.mul(t2[:, fs:fs + F], xt[2][:, fs:fs + F],
                                  w[:, 2 * T + t:2 * T + t + 1])
                    nc.scalar.mul(t3[:, fs:fs + F], xt[3][:, fs:fs + F],
                                  w[:, 3 * T + t:3 * T + t + 1])
                nc.gpsimd.tensor_tensor(out=acc1[:, :], in0=t01[:, :], in1=t2[:, :], op=add)
                nc.gpsimd.tensor_tensor(out=acc[:, :], in0=acc1[:, :], in1=t3[:, :], op=add)

                dst = out[lo:hi, :].rearrange("(p t) f -> p (t f)", p=128)
                nc.sync.dma_start(out=dst, in_=acc[:, :])

    nc.compile()
    return nc


def kernel(input, a, _trace=False):
    input = np.ascontiguousarray(np.asarray(input, dtype=np.float32))
    a = np.asarray(a, dtype=np.float32).reshape(F)

    if "nc" not in _CACHE:
        _CACHE["nc"] = _build_kernel()
    nc = _CACHE["nc"]

    pad = N_PAD - input.shape[1]
    xp = np.concatenate(
        [input, np.zeros((NMETA, pad, F), np.float32)], axis=1
    ) if pad else input

    a_rep = np.tile(a[None, :], (128, T)).astype(np.float32)

    in_maps = []
    for c in range(N_CORES):
        sl = xp[:, c * NC_NODES:(c + 1) * NC_NODES, :]
        in_maps.append({"input": np.ascontiguousarray(sl), "a_rep": a_rep})

    res = bass_utils.run_bass_kernel_spmd(
        nc, in_maps, core_ids=list(range(N_CORES)), trace=_trace
    )
    outs = [res.results[c]["out"] for c in range(N_CORES)]
    full = np.concatenate(outs, axis=0)[:N_FULL]
    if _trace:
        return full, res
    return full



# revision 3
# speedup vs baseline: 1.4421x; 1.4421x over previous
"""MetapathAggrLayer Trainium2 kernel — v3 (big-op restructure).

Per node n: e_m = leakyrelu(x[m,n,:].a), w = softmax(e), out = sum_m w_m x[m,n,:].
Data-parallel over N across 8 NeuronCores; nodes-on-partitions layout.

v3 vs v2: one merged DMA + one merged prefix-scan per macro-tile (all 4
metapaths), scores via prefix differences fused with leakyrelu in one custom
DVE op, weighted sum via broadcast-AP tensor_tensor big ops (per-(node,chunk)
weights broadcast along F with a stride-0 inner dim) split across
Vector/Scalar/GpSimd, fp16 product/accumulate path with a casting SWDGE
output DMA.
"""

import sys

sys.path.insert(0, "/opt/trn_rl_repo")

import numpy as np

import concourse.bacc as bacc
import concourse.mybir as mybir
from concourse import bass_utils, dve_ops
from concourse.dve_spec import Spec, Src0, Src1, C0, scan, maxx, AluOp, lower, _has_src1
from concourse.dve_uop import DveOpSpec
from concourse.tile import TileContext

ALPHA = 0.2
NMETA = 4
F = 64
N_FULL = 1_000_000
N_CORES = 8
T = 32                     # nodes per partition per macro-tile
NODES_PER_MACRO = 128 * T  # 4096
MACROS_PER_CORE = 31
NC_NODES = MACROS_PER_CORE * NODES_PER_MACRO  # 126_976
N_PAD = N_CORES * NC_NODES                    # 1_015_808
NSEG = NMETA * T           # score segments per partition per macro
NM = T * F                 # per-metapath free elems
NALL = NMETA * NM          # merged free elems

_CACHE = {}


def _register_op(name, spec, subdim=False):
    if name in dve_ops._SUB_OPCODE_FOR_NAME:
        return next(o for o in dve_ops.OPS if o.name == name)
    row = dve_ops._CUSTOM_DVE_ROW_BASE + len(dve_ops.OPS)
    assert row < 0x20
    shas = {}
    for ver in ("v3", "v4"):
        s = DveOpSpec(name=name, opcode=row, uops=lower(spec, ver=ver),
                      rd1_en=_has_src1(spec))
        shas[ver] = s.sha(ver)
    op = dve_ops.DveOp(name, spec, subdim, shas)
    dve_ops.OPS.append(op)
    dve_ops.CUSTOM_DVE_SPECS[name] = spec
    dve_ops._SUB_OPCODE_FOR_NAME[name] = row
    return op


def _get_ops():
    scan_mul = _register_op(
        "MPA_SCAN_MUL",
        Spec(
            body=scan(AluOp.ADD, Src0 * Src1),
            reference=lambda in0, in1, s0, s1: np.cumsum(
                (in0.astype(np.float32) * in1.astype(np.float32)), axis=-1
            ),
        ),
    )
    ext_lrelu = _register_op(
        "MPA_EXT_LRELU",
        Spec(
            body=(lambda d: maxx(d, d * C0))(Src0 - Src1),
            reference=lambda in0, in1, s0, s1: np.maximum(in0 - in1, (in0 - in1) * s0),
        ),
    )
    return scan_mul, ext_lrelu


def _build_kernel():
    scan_mul, ext_lrelu = _get_ops()

    nc = bacc.Bacc("TRN2", target_bir_lowering=False, debug=False)
    f32 = mybir.dt.float32
    f16 = mybir.dt.float16

    x_in = nc.dram_tensor("input", (NMETA, NC_NODES, F), f32, kind="ExternalInput").ap()
    a_rep_in = nc.dram_tensor("a_rep", (128, NM), f32, kind="ExternalInput").ap()
    out = nc.dram_tensor("out", (NC_NODES, F), f32, kind="ExternalOutput").ap()

    mult = mybir.AluOpType.mult
    add = mybir.AluOpType.add

    with TileContext(nc) as tc:
        with tc.tile_pool(name="const", bufs=1) as cpool, \
             tc.tile_pool(name="xp", bufs=2) as xpool, \
             tc.tile_pool(name="pp", bufs=1) as ppool, \
             tc.tile_pool(name="fp", bufs=2) as fpool, \
             tc.tile_pool(name="small", bufs=2) as spool:
            a_rep = cpool.tile([128, NM], f32)
            nc.sync.dma_start(out=a_rep[:, :], in_=a_rep_in)
            a_bc = a_rep[:, :].rearrange("p (o n) -> p o n", o=1).broadcast_to(
                [128, NMETA, NM])

            for i in range(MACROS_PER_CORE):
                lo = i * NODES_PER_MACRO
                hi = lo + NODES_PER_MACRO

                # ---- merged load: [128, (m t f)]
                xm = xpool.tile([128, NALL], f32, tag="x")
                src = x_in[:, lo:hi, :].rearrange("m (p t) f -> p m t f", p=128)
                dst4 = xm[:, :].rearrange("p (m t f) -> p m t f", m=NMETA, f=F)
                nc.sync.dma_start(out=dst4, in_=src)

                # ---- scores: one prefix scan of x*a over the merged row;
                # segment sums recovered as boundary differences fused with
                # leakyrelu.
                P = ppool.tile([128, NALL + 1], f32, tag="P")
                nc.gpsimd.memset(P[:, 0:1], 0.0)
                nc.vector._custom_dve(
                    scan_mul, out=P[:, 1:NALL + 1], in0=xm[:, :], in1=a_bc,
                )
                p3 = P[:, 1:NALL + 1].rearrange("p (s f) -> p s f", f=F)
                p_hi = p3[:, :, F - 1:F]
                p_lo = P[:, 0:NALL].rearrange("p (s f) -> p s f", f=F)[:, :, 0:1]
                e = spool.tile([128, NSEG], f32, tag="e")
                nc.vector._custom_dve(
                    ext_lrelu, out=e[:, :], in0=p_hi, in1=p_lo, s0=ALPHA,
                )

                # ---- softmax over metapaths (m-major segment layout)
                u = spool.tile([128, NSEG], f32, tag="u")
                nc.scalar.activation(u[:, :], e[:, :],
                                     mybir.ActivationFunctionType.Exp)
                h = spool.tile([128, 2 * T], f32, tag="h")
                nc.vector.tensor_tensor(out=h[:, :], in0=u[:, 0:2 * T],
                                        in1=u[:, 2 * T:4 * T], op=add)
                s = spool.tile([128, T], f32, tag="s")
                nc.vector.tensor_tensor(out=s[:, :], in0=h[:, 0:T],
                                        in1=h[:, T:2 * T], op=add)
                r = spool.tile([128, T], f32, tag="r")
                nc.vector.reciprocal(r[:, :], s[:, :])
                w = spool.tile([128, NSEG], f32, tag="w")
                r_bc = r[:, :].rearrange("p (o t) -> p o t", o=1).broadcast_to(
                    [128, NMETA, T])
                nc.vector.tensor_tensor(
                    out=w[:, :].rearrange("p (m t) -> p m t", m=NMETA),
                    in0=u[:, :].rearrange("p (m t) -> p m t", m=NMETA),
                    in1=r_bc, op=mult)

                # ---- weighted sum: per-(node,t) weight broadcast along F.
                # m0,m1 on Vector; m2 on Scalar (per-t scale loop); m3 on GpSimd.
                def wb(m):
                    return w[:, m * T:(m + 1) * T].rearrange(
                        "p (t o) -> p t o", o=1).broadcast_to([128, T, F])

                def x3(m):
                    return xm[:, m * NM:(m + 1) * NM].rearrange(
                        "p (t f) -> p t f", f=F)

                t0 = fpool.tile([128, NM], f16, tag="t0")
                t1 = fpool.tile([128, NM], f16, tag="t1")
                t2 = fpool.tile([128, NM], f16, tag="t2")
                t3 = fpool.tile([128, NM], f16, tag="t3")
                nc.vector.tensor_tensor(
                    out=t0[:, :].rearrange("p (t f) -> p t f", f=F),
                    in0=x3(0), in1=wb(0), op=mult)
                nc.vector.tensor_tensor(
                    out=t1[:, :].rearrange("p (t f) -> p t f", f=F),
                    in0=x3(1), in1=wb(1), op=mult)
                for t in range(T):
                    fs = t * F
                    nc.scalar.mul(t2[:, fs:fs + F], xm[:, 2 * NM + fs:2 * NM + fs + F],
                                  w[:, 2 * T + t:2 * T + t + 1])
                nc.gpsimd.tensor_tensor(
                    out=t3[:, :].rearrange("p (t f) -> p t f", f=F),
                    in0=x3(3), in1=wb(3), op=mult)

                a01 = fpool.tile([128, NM], f16, tag="a01")
                a23 = fpool.tile([128, NM], f16, tag="a23")
                acc = fpool.tile([128, NM], f16, tag="acc")
                nc.vector.tensor_tensor(out=a01[:, :], in0=t0[:, :], in1=t1[:, :],
                                        op=add)
                nc.gpsimd.tensor_tensor(out=a23[:, :], in0=t2[:, :], in1=t3[:, :],
                                        op=add)
                nc.vector.tensor_tensor(out=acc[:, :], in0=a01[:, :], in1=a23[:, :],
                                        op=add)

                # ---- store with fp16 -> fp32 cast (SWDGE)
                dst = out[lo:hi, :].rearrange("(p t) f -> p (t f)", p=128)
                nc.gpsimd.dma_start(out=dst, in_=acc[:, :])

    nc.compile()
    return nc


def kernel(input, a, _trace=False):
    input = np.ascontiguousarray(np.asarray(input, dtype=np.float32))
    a = np.asarray(a, dtype=np.float32).reshape(F)

    if "nc" not in _CACHE:
        _CACHE["nc"] = _build_kernel()
    nc = _CACHE["nc"]

    pad = N_PAD - input.shape[1]
    xp = np.concatenate(
        [input, np.zeros((NMETA, pad, F), np.float32)], axis=1
    ) if pad else input

    a_rep = np.tile(a[None, :], (128, T)).astype(np.float32)

    in_maps = []
    for c in range(N_CORES):
        sl = xp[:, c * NC_NODES:(c + 1) * NC_NODES, :]
        in_maps.append({"input": np.ascontiguousarray(sl), "a_rep": a_rep})

    res = bass_utils.run_bass_kernel_spmd(
        nc, in_maps, core_ids=list(range(N_CORES)), trace=_trace
    )
    outs = [res.results[c]["out"] for c in range(N_CORES)]
    full = np.concatenate(outs, axis=0)[:N_FULL]
    if _trace:
        return full, res
    return full
